# revision 57
# baseline (speedup 1.0000x reference)
"""MMD loss kernel for Trainium2 (8 NeuronCores, Bass/Tile).

reference math:
  src = X[:2048], tgt = X[2048:],  D=512
  xx = mean over [4096,4096] of sum_k exp(-d2_dup(src,src)/(bw_xx*2^k))
  (dup matrix mean == mean over the 2048^2 block), similarly yy, and
  xy uses the full 4096^2 matrix of X.
  bw for (a,b) = sum(d2([a;b]))/(m^2-m) / mul^(num//2),  mul=2, num=5.

Strategy:
  - bandwidth sums have a closed form: sum_block d2 = 2n*sum(sq) - 2|sum x|^2
    -> computed host-side in fp64, passed to the device as runtime
    activation *scales* (per-partition AP), so no first pass over d2.
  - pairwise tile: PSUM M = G - sq_i/2 - sq_j/2 = -d2/2 via an augmented
    matmul: G from a SINGLE bf16 matmul per k-chunk (input rounding costs
    ~2e-4 final rel err, measured), plus K=4 aug rows carrying a bf16
    hi/lo split of -sq/2 (the sq term stays near-fp32 exact).
  - 5-kernel sum: u = exp(scale*M) with scale = 1/(8*bw_base); then 4
    squarings give the other 4 kernels (fp16 chain, ~2e-3 rel err).
  - squares run in one of four modes, statically planned (_plan_schedule,
    a quota-constrained randomized list scheduler whose emission order
    drives the Tile scheduler's priorities):
      A  ACT Square w/ fused accum rider      (2.08us)
      D  DVE scalar_tensor_tensor w/ accum    (2.19us)
      Db DVE plain tensor_tensor, fp16 2x     (1.22us) + PE sums
      Pb Pool plain tensor_tensor             (4.2us)  + PE sums
    Pool cannot run accum-fused ops on real HW (NCC rejects STT on Pool),
    so bundle (Db/Pb) sums ride the otherwise-idle PE: ones/weight-matmuls
    accumulate w_c * colsums into a dedicated [2,512] f32 PSUM bank
    (row 0 = own chain, row 1 = xy chain), copied out once at the end.
  - two half-chunk PSUM tiles (2 banks each; ps0 double-, ps1 single-
    buffered - the accumulator takes the 8th bank) keep the PE streaming;
    ~36 tiny warm-up matmuls during the input DMA hold the PE HAM window
    busy so real matmuls start at 2.4 GHz.
  - symmetry: the distance matrix is symmetric. Own-half blocks use cyclic
    coverage (each 512-row core covers col-groups k,k+1,k+2 with weights
    1,2,1); cross src/tgt blocks are covered once with weight 2 across the
    8 cores. Every core runs the SAME program on a per-core permuted
    column layout: local cols = [own(k), own(k+1), own(k+2), cross0, cross1]
    (2560 of 4096 columns).
  - LOOP > 1 wraps the body in a hardware For_i (timing builds only).
"""

import sys

sys.path.insert(0, "/opt/trn_rl_repo")

import numpy as np
import ml_dtypes

N, D, HALF, BLK = 4096, 512, 2048, 512
NCORES = 8
NCHUNK = 5          # local col chunks of 512: 3 own (w 1,2,1) + 2 cross (w 2)
CHUNK_W = [1, 2, 1, 2, 2]
NPASS = 5           # exp + 4 squares
RID_W = 6           # rider slots per unit: exp A-half, exp B-half, 4 squares
LC = NCHUNK * 512   # 2560 local columns

MM_DT = "bfloat16"
CH_DT = "float16"   # chain (exp/square) dtype; 16-bit enables 2x DVE tensor_tensor

REPEAT = 1          # python-unrolled body repeats (distinct rider outputs)
LOOP = 1            # hardware-loop trip count (timing builds; riders idempotent)


def _schedule():
    """Static (core-independent) unit schedule: (chunk, chain)."""
    sched = []
    for c in range(NCHUNK):
        chains = ("own", "xy") if c < 3 else ("xy",)
        for chain in chains:
            sched.append((c, chain))
    return sched


SCHED = _schedule()
NUNIT = len(SCHED)  # 8


def _local_cols(core):
    half, k = core // 4, core % 4
    own_base, other_base = half * HALF, (1 - half) * HALF
    groups = [k, (k + 1) % 4, (k + 2) % 4]
    cols = [own_base + 512 * g + np.arange(512) for g in groups]
    if half == 0:
        cross = [0, 1] if k % 2 == 0 else [2, 3]
    else:
        cross = [1, 3] if k < 2 else [0, 2]
    cols += [other_base + 512 * b + np.arange(512) for b in cross]
    return np.concatenate(cols)


def _plan_schedule():
    """Greedy list-schedule of one body: returns ops in planned start order.

    Square engines are chosen dynamically (earliest finish) from:
      A  - ACT Square, fused accum rider
      D  - DVE scalar_tensor_tensor, fused accum rider
      Db - DVE plain tensor_tensor (2x fp16) + PE ones-matmul sums
      Pb - Pool plain tensor_tensor + PE ones-matmul sums
    Bundle sums ride the idle PE into a dedicated PSUM accumulator.
    The emission order becomes the Tile scheduler's priority order.
    """
    COST = {"mm": 213, "exp": 1225, "A": 2079, "D": 2194,
            "Db": 1224, "Pb": 4400, "sum": 950}
    # per-kind square quotas: best of an offline sweep (planned span 42.4us)
    QUOTA = globals().get("QUOTA_OVERRIDE") or {"A": 8, "D": 10, "Db": 8,
                                               "Pb": 6}
    by_chunk = {}
    for u, (c, chain) in enumerate(SCHED):
        by_chunk.setdefault(c, []).append(u)

    ops = {}           # id -> dict

    def add(oid, kind, engine, cost, deps, c=None, u=None, h=None, lvl=None):
        ops[oid] = dict(kind=kind, engine=engine, cost=cost, deps=list(deps),
                        c=c, u=u, h=h, lvl=lvl)

    for c in range(NCHUNK):
        dma_ready = (2600.0 if c == 0 else 3600.0 + 1500.0 * (c - 1))
        for h in range(2):
            mdeps = []
            # PSUM rotation: ps0 double-buffered (2-chunk distance), ps1
            # single-buffered (acc bank takes its second buffer)
            if h == 0 and c >= 2:
                mdeps = [f"exp_{c - 2}_{u}_{h}" for u in by_chunk[c - 2]]
            if h == 1 and c >= 1:
                mdeps = [f"exp_{c - 1}_{u}_{h}" for u in by_chunk[c - 1]]
            add(f"mm_{c}_{h}", "mm", "PE", 10 * COST["mm"], mdeps, c=c, h=h)
            ops[f"mm_{c}_{h}"]["ready"] = dma_ready
        for u in by_chunk[c]:
            for h in range(2):
                add(f"exp_{c}_{u}_{h}", "exp", "A", COST["exp"],
                    [f"mm_{c}_{h}"], c=c, u=u, h=h, lvl=0)
        for lvl in range(1, 5):
            for u in by_chunk[c]:
                deps = ([f"exp_{c}_{u}_0", f"exp_{c}_{u}_1"] if lvl == 1
                        else [f"sq_{c}_{u}_{lvl - 1}"])
                add(f"sq_{c}_{u}_{lvl}", "sq", None, None, deps,
                    c=c, u=u, lvl=lvl)

    ENG_OF = {"A": "A", "D": "D", "Db": "D", "Pb": "P"}
    base_ops = dict(ops)

    def run_once(rng, slack):
        import copy
        lops = copy.deepcopy(base_ops)
        eng_free = {"PE": 2400.0, "A": 0.0, "D": 0.0, "P": 0.0}
        done, order = {}, []
        unsched = set(lops)
        quota = dict(QUOTA)
        while unsched:
            best = None
            for oid in unsched:
                o = lops[oid]
                if any(d not in done for d in o["deps"]):
                    continue
                ready = max([done[d] + 250.0 for d in o["deps"]]
                            + [o.get("ready", 0.0)])
                if o["kind"] == "sq":
                    kinds = [k for k in ("A", "D", "Db", "Pb") if quota[k] > 0]
                    cands = sorted(
                        (max(ready, eng_free[ENG_OF[k]]) + COST[k], k)
                        for k in kinds)
                    near = [cd for cd in cands if cd[0] <= cands[0][0] + slack]
                    fin, kind = near[rng.randrange(len(near))]
                    st, eng = fin - COST[kind], kind
                else:
                    eng = o["engine"]
                    st = max(ready, eng_free[eng])
                    fin = st + o["cost"]
                key = (st, fin, oid)
                if best is None or key < best[0]:
                    best = (key, oid, eng, st, fin)
            _, oid, eng, st, fin = best
            o = lops[oid]
            o["engine"] = eng
            done[oid] = fin
            eng_free[ENG_OF.get(eng, eng)] = fin
            unsched.discard(oid)
            order.append(oid)
            if o["kind"] == "sq":
                quota[eng] -= 1
                if eng in ("Db", "Pb"):
                    sid = f"sum_{o['c']}_{o['u']}_{o['lvl']}"
                    lops[sid] = dict(kind="sum", engine="PE",
                                     cost=COST["sum"], deps=[oid],
                                     c=o["c"], u=o["u"], h=None, lvl=o["lvl"])
                    unsched.add(sid)
        return order, lops, max(done.values())

    import random
    best = None
    for trial in range(120):
        rng = random.Random(1234 + trial)
        slack = [0.0, 400.0, 800.0, 1600.0][trial % 4]
        res = run_once(rng, slack)
        if best is None or res[2] < best[2]:
            best = res
    return best


_PLAN = None


def _get_plan():
    global _PLAN
    if _PLAN is None:
        _PLAN = _plan_schedule()
    return _PLAN


def _build_program():
    import concourse.bacc as bacc
    import concourse.mybir as mybir
    import concourse.tile as tile

    f32 = mybir.dt.float32
    mm_dt = getattr(mybir.dt, MM_DT)
    ch_dt = getattr(mybir.dt, CH_DT, mm_dt)
    nrep = globals().get("REPEAT", 1)
    nloop = globals().get("LOOP", 1)

    nc = bacc.Bacc("TRN2", target_bir_lowering=False, debug=False,
                   num_devices=NCORES)
    xt_d = nc.dram_tensor("xt", [NCHUNK, 128, 4, 512], mm_dt,
                          kind="ExternalInput")
    aug_d = nc.dram_tensor("aug", [4, LC + 512], mm_dt, kind="ExternalInput")
    sc_d = nc.dram_tensor("scales", [128, 2], f32, kind="ExternalInput")
    ws_d = nc.dram_tensor("wsel", [128, 8], ch_dt, kind="ExternalInput")
    rid_d = nc.dram_tensor("riders", [nrep * NUNIT, 128, RID_W], f32,
                           kind="ExternalOutput")
    acc_d = nc.dram_tensor("accv", [2, 512], f32, kind="ExternalOutput")

    by_chunk = {}
    for u, (c, chain) in enumerate(SCHED):
        by_chunk.setdefault(c, []).append((u, chain))

    with tile.TileContext(nc) as tc:
        with (
            tc.tile_pool(name="xtp", bufs=1) as xtp,
            tc.tile_pool(name="augp", bufs=1) as augp,
            tc.tile_pool(name="scp", bufs=1) as scp,
            tc.tile_pool(name="ridp", bufs=1) as ridp,
            tc.tile_pool(name="psp", bufs=8, space="PSUM") as psp,
            tc.tile_pool(name="up", bufs=2) as up,
        ):
            xt3 = xtp.tile([128, 4, LC], mm_dt, tag="xt", name="xt")
            aug = augp.tile([4, LC + 512], mm_dt, tag="aug", name="aug")
            sc = scp.tile([128, 2], f32, tag="sc", name="sc")
            ws = scp.tile([128, 8], ch_dt, tag="ws", name="ws")
            # sc (tiny) first: the PE warm-up matmuls read it. Then chunk-0
            # columns and aug so real matmuls start early; rest follows.
            nc.sync.dma_start(out=sc[:], in_=sc_d.ap())
            # chunk 0 split across both HWDGE queues (SP + ACT) to halve its
            # landing time; aug rides the ACT queue too (needed ~1 chunk in)
            nc.sync.dma_start(out=xt3[0:64, :, 0:512], in_=xt_d.ap()[0][0:64])
            nc.scalar.dma_start(out=xt3[64:128, :, 0:512],
                                in_=xt_d.ap()[0][64:128])
            nc.scalar.dma_start(out=aug[:], in_=aug_d.ap())
            for c in range(1, NCHUNK):
                nc.sync.dma_start(out=xt3[:, :, 512 * c:512 * c + 512],
                                  in_=xt_d.ap()[c])
                if c == 1:
                    nc.scalar.dma_start(out=ws[:], in_=ws_d.ap())

            riders = [[ridp.tile([128, RID_W], f32, tag=f"rid{u}_{rp}",
                                 name=f"rid{u}_{rp}") for u in range(NUNIT)]
                      for rp in range(nrep)]
            for rp in range(nrep):
                for u in range(NUNIT):
                    nc.gpsimd.memset(riders[rp][u][:], 0.0)
            accp = psp.tile([2, 512], f32, tag="acc", name="acc", bufs=1)
            acc_sb = ridp.tile([2, 512], f32, tag="accsb", name="accsb")

            # PE warm-up: tiny matmuls on the (first-arriving) sc tile keep
            # the PE HAM activity window busy during the xt DMA so the real
            # matmuls start at full clock.
            warm_ps = psp.tile([128, 1024], f32, tag="ps0", name="warm_ps",
                               bufs=2)
            for w in range(36):
                nc.tensor.matmul(out=warm_ps[0:1, 0:2],
                                 lhsT=sc[:, 0:1], rhs=sc[:, 0:2],
                                 start=True, stop=True)

            order, plan_ops, _ = _get_plan()
            n_sums = sum(1 for oid in order
                         if plan_ops[oid]["kind"] == "sum")

            def body(rep):
                psh = {}
                cur = {}
                sum_i = [0]
                for oid in order:
                    o = plan_ops[oid]
                    c, u, h, lvl = o["c"], o["u"], o["h"], o["lvl"]
                    if o["kind"] == "sum":
                        chain = SCHED[u][1]
                        v = (0 if chain == "own" else 2) + (CHUNK_W[c] - 1)
                        src = cur[(c, u, lvl)]
                        for j in range(4):
                            first = sum_i[0] == 0
                            last = sum_i[0] == 4 * n_sums - 1
                            nc.tensor.matmul(
                                out=accp[:],
                                lhsT=ws[:, 2 * v:2 * v + 2],
                                rhs=src[:, 512 * j:512 * j + 512],
                                start=first, stop=last)
                            sum_i[0] += 1
                        continue
                    if o["kind"] == "mm":
                        p = psp.tile([128, 1024], f32, tag=f"ps{h}",
                                     name=f"ps{c}_{h}",
                                     bufs=2 if h == 0 else 1)
                        psh[(c, h)] = p
                        for s2 in range(2):
                            s = 2 * h + s2
                            pss = p[:, 512 * s2:512 * s2 + 512]
                            for k in range(4):
                                nc.tensor.matmul(
                                    out=pss,
                                    lhsT=xt3[:, k, 128 * s:128 * s + 128],
                                    rhs=xt3[:, k, 512 * c:512 * c + 512],
                                    start=(k == 0), stop=False)
                            nc.tensor.matmul(
                                out=pss,
                                lhsT=aug[:, LC + 128 * s:LC + 128 * s + 128],
                                rhs=aug[:, 512 * c:512 * c + 512],
                                start=False, stop=True)
                    elif o["kind"] == "exp":
                        rid = riders[rep][u]
                        sci = 0 if SCHED[u][1] == "own" else 1
                        if (c, u, 0) not in cur:
                            cur[(c, u, 0)] = up.tile([128, 2048], ch_dt,
                                                     tag=f"u{u % 2}",
                                                     name=f"u{u}_0", bufs=6)
                        t = cur[(c, u, 0)]
                        nc.scalar.activation(
                            out=t[:, 1024 * h:1024 * h + 1024],
                            in_=psh[(c, h)][:],
                            func=mybir.ActivationFunctionType.Exp,
                            scale=sc[:, sci:sci + 1],
                            accum_out=rid[:, h:h + 1])
                    else:  # sq
                        rid = riders[rep][u]
                        prev = cur[(c, u, lvl - 1)]
                        nxt = up.tile([128, 2048], ch_dt, tag=f"u{u % 2}",
                                      name=f"u{u}_{lvl}", bufs=6)
                        ao = rid[:, lvl + 1:lvl + 2]
                        eng = o["engine"]
                        if eng == "A":
                            nc.scalar.activation(
                                out=nxt[:], in_=prev[:],
                                func=mybir.ActivationFunctionType.Square,
                                accum_out=ao)
                        elif eng == "D":
                            nc.vector.scalar_tensor_tensor(
                                out=nxt[:], in0=prev[:], scalar=1.0,
                                in1=prev[:],
                                op0=mybir.AluOpType.mult,
                                op1=mybir.AluOpType.mult,
                                accum_out=ao)
                        else:  # Db / Pb: plain tensor_tensor, sum on PE
                            e = nc.vector if eng == "Db" else nc.gpsimd
                            e.tensor_tensor(
                                out=nxt[:], in0=prev[:], in1=prev[:],
                                op=mybir.AluOpType.mult)
                        cur[(c, u, lvl)] = nxt
                        if lvl == 4:
                            nc.sync.dma_start(
                                out=rid_d.ap()[rep * NUNIT + u],
                                in_=riders[rep][u][:])

            if nloop > 1:
                with tc.For_i(0, nloop, 1):
                    body(0)
            else:
                for rep in range(nrep):
                    body(rep)
            nc.vector.tensor_copy(out=acc_sb[:], in_=accp[:])
            nc.sync.dma_start(out=acc_d.ap(), in_=acc_sb[:])

    nc.compile()
    return nc


_PROG = None


def _get_program():
    global _PROG
    if _PROG is None:
        _PROG = _build_program()
    return _PROG


def _prep_inputs(latent):
    X = np.asarray(latent, np.float32)
    X64 = X.astype(np.float64)
    sq = (X64 * X64).sum(1)                      # [N]
    M2 = float(N) * N - N

    def block_d2_sum(lo, hi):
        n = hi - lo
        sv = X64[lo:hi].sum(0)
        return 2.0 * (n * sq[lo:hi].sum()) - 2.0 * (sv @ sv)

    S_src = block_d2_sum(0, HALF)
    S_tgt = block_d2_sum(HALF, N)
    sv_all = X64.sum(0)
    S_full = 2.0 * (N * sq.sum()) - 2.0 * (sv_all @ sv_all)

    bw_xx = S_src / M2           # dup-matrix 4x cancels the /mul^(num//2)
    bw_yy = S_tgt / M2
    bw_xy = (S_full / M2) / 4.0

    in_maps = []
    for core in range(NCORES):
        lc = _local_cols(core)
        xf = X[lc].T.reshape(4, 128, NCHUNK, 512)
        xt = np.ascontiguousarray(xf.transpose(2, 1, 0, 3)).astype(
            ml_dtypes.bfloat16)                  # [NCHUNK, 128, 4, 512]
        sql = sq[lc]
        v = -0.5 * sql
        hi = np.asarray(v, ml_dtypes.bfloat16).astype(np.float64)
        lo = (v - hi).astype(np.float32)
        hi = hi.astype(np.float32)
        ones = np.ones_like(hi)
        aug = np.zeros((4, LC + 512), ml_dtypes.bfloat16)
        aug[0, :LC] = hi
        aug[1, :LC] = lo
        aug[2, :LC] = ones
        aug[3, :LC] = ones
        aug[0, LC:] = 1.0
        aug[1, LC:] = 1.0
        aug[2, LC:] = hi[:512]
        aug[3, LC:] = lo[:512]

        bw_own = bw_xx if core < 4 else bw_yy
        scales = np.zeros((128, 2), np.float32)
        scales[:, 0] = 1.0 / (8.0 * bw_own)
        scales[:, 1] = 1.0 / (8.0 * bw_xy)
        # PE-sum lhsT variants (own,w1),(own,w2),(xy,w1),(xy,w2): col 0
        # accumulates into acc row 0 (own), col 1 into row 1 (xy); the
        # chunk weight is folded into the value.
        wsel = np.zeros((128, 8), np.float16 if CH_DT == "float16"
                        else ml_dtypes.bfloat16)
        for v, (ch_col, w) in enumerate([(0, 1), (0, 2), (1, 1), (1, 2)]):
            wsel[:, 2 * v + ch_col] = w
        in_maps.append({"xt": xt, "aug": aug, "scales": scales,
                        "wsel": wsel})
    return in_maps


def _postprocess(results):
    # which rider slots carry fused accums (bundle squares sum via accv)
    order, plan_ops, _ = _get_plan()
    fused = {}  # (u, lvl) -> True if rider slot lvl+1 is valid
    for oid in order:
        o = plan_ops[oid]
        if o["kind"] == "sq":
            fused[(o["u"], o["lvl"])] = o["engine"] in ("A", "D")
    S_own = np.zeros(NCORES)
    S_xy = np.zeros(NCORES)
    for core in range(NCORES):
        r = results[core]["riders"].astype(np.float64)  # [NUNIT,128,RID_W]
        acc = results[core]["accv"].astype(np.float64)  # [2,512]
        for u, (c, chain) in enumerate(SCHED):
            val = r[u, :, 0:2].sum()
            for lvl in range(1, 5):
                if fused[(u, lvl)]:
                    val += r[u, :, lvl + 1].sum()
            val *= CHUNK_W[c]
            if chain == "own":
                S_own[core] += val
            else:
                S_xy[core] += val
        S_own[core] += acc[0].sum()
        S_xy[core] += acc[1].sum()
    xx = S_own[:4].sum() / (HALF * HALF)
    yy = S_own[4:].sum() / (HALF * HALF)
    xy = S_xy.sum() / (float(N) * N)
    return np.float32(xx + yy - 2.0 * xy)


def _run(inputs, trace=False, **kw):
    from concourse.bass_utils import run_bass_kernel_spmd
    nc = _get_program()
    in_maps = _prep_inputs(inputs["latent"])
    res = run_bass_kernel_spmd(nc, in_maps, list(range(NCORES)),
                               trace=trace, **kw)
    return _postprocess(res.results), res


def kernel(**inputs):
    out, _ = _run(inputs, trace=False)
    return out


if __name__ == "__main__":
    rng = np.random.default_rng(0)
    lat = rng.standard_normal((N, D)).astype(np.float32)
    print(kernel(latent=lat,
                 domain=np.concatenate([np.zeros(HALF, np.int32),
                                        np.ones(HALF, np.int32)])))
